# revision 24
# baseline (speedup 1.0000x reference)
"""Trainium2 Bass kernel for the windowed 3-channel MLP (dense_mlp).

Reference computation (B=8192):
  x [B, 6144] -> view [B, 3, 2048]
  16 overlapping windows/channel (len 256, stride 119)
  h[b,c,w,:] = win @ W1[c,w] + b1[c,w]          # [B,3,16,64]
  h = mean over c                               # [B,16,64]
  g[b,grp]   = h-grp(4 windows=256) @ W2[grp] + b2   # [B,4,64]
  out        = g.reshape(B,256) @ W3 + b3       # [B,255]

Strategy: pure data parallelism over 8 cores (B/8 = 1024 rows each).
Compute in fp16 (accumulation in f32 PSUM).  x is repacked on the HOST into
the exact SBUF layout the kernel wants -- chunk-major [2][48 ktiles][128][512]
fp16 with k-tiles interleaved [t][c] -- so every device load is a plain
contiguous-burst DMA over a dense DRAM block (full HBM row utilization) and
window-pair p's tiles form a contiguous range (staged sub-DMAs let layer-1
matmuls start after ~20% of the chunk load has landed).

On-device per core:
  - 2 batch chunks of 512 (PSUM-bank-limited matmul free dim, minimizes
    per-instruction overhead).
  - Layer 1 as banded matmuls over 128-aligned k-tiles with host-packed
    zero-padded weight blocks (channel-mean folded into PSUM accumulation,
    1/3 folded into W1).
  - Layers 2/3 stay feature-major; layer 3 uses gT as lhsT so the output
    comes out batch-major for a contiguous DMA out.
  - The timing loop (reps>1) is unrolled 4x inside tc.For_i to amortize the
    all-engine back-edge barrier.
"""

import sys

sys.path.insert(0, "/opt/trn_rl_repo")

import numpy as np

import concourse.bass as bass
import concourse.mybir as mybir
import concourse.tile as tile
from concourse import bacc
from concourse.bass_utils import run_bass_kernel_spmd

P = 128
N_CORES = 8
B_FULL = 8192
B_SHARD = B_FULL // N_CORES          # 1024
CH_LEN = 2048
N_CH = 3
K_FULL = N_CH * CH_LEN               # 6144
N_WIN = 16
WIN = 256
STRIDE = 119
N_PAIR = 8                           # window pairs (2 windows x 64 = 128 feats)
KT_CH = CH_LEN // P                  # 16 k-tiles per channel
KT_ALL = K_FULL // P                 # 48
NB = 512                             # batch chunk (matmul free dim = PSUM bank)
N_CHUNK = B_SHARD // NB              # 2
N_OUT = 255

# k-tile-group ranges (in q = t*3+c units) for the staged sub-DMAs.
# Pair m needs q in [3*lo(m), 3*hi(m)+2]:
#   p0<-A, p1<-B, p2/p3<-C, p4/p5<-D, p6/p7<-E
QGROUPS = [(0, 9), (9, 15), (15, 27), (27, 39), (39, 48)]


def _pair_tiles(m):
    """k-tiles of one channel that intersect window pair m (rows 238m..238m+374)."""
    lo = (2 * STRIDE * m) // P
    hi = (2 * STRIDE * m + 2 * STRIDE + WIN - 2 - STRIDE) // P  # (238m+374)//128
    return list(range(lo, min(hi, KT_CH - 1) + 1))

# Block order for layer-1 packed weights: for m, for c, for t.
BLOCKS = [(m, c, t) for m in range(N_PAIR) for c in range(N_CH) for t in _pair_tiles(m)]
BLK_IDX = {key: i for i, key in enumerate(BLOCKS)}
N_BLK = len(BLOCKS)                  # 90


def _pack_weights(W1, b1, W2, b2, W3, b3):
    """Host-side packing of the tiny weight tensors into device layouts."""
    W1 = np.asarray(W1, dtype=np.float32)
    ki = np.arange(P)[:, None]                    # tile-local k row
    j = np.arange(P)[None, :]                     # pair-local output feature
    w_off = j // 64                               # window within pair
    n = j % 64

    w1p = np.zeros((N_BLK, P, P), dtype=np.float32)
    for i, (m, c, t) in enumerate(BLOCKS):
        w = 2 * m + w_off                         # [1,128] window index
        koff = P * t + ki - STRIDE * w            # [128,128] k within window
        mask = (koff >= 0) & (koff < WIN)
        w1p[i] = np.where(
            mask, W1[c, w, np.clip(koff, 0, WIN - 1), n] / 3.0, 0.0
        )
    # device layout: [P(ki), N_BLK * P(j)] contiguous per partition
    w1sb = np.ascontiguousarray(
        w1p.transpose(1, 0, 2).reshape(P, N_BLK * P)
    ).astype(np.float16)

    # W2 [4,256,64] -> pieces [g,p][128,64] -> [P, 8, 64]
    w2p = np.asarray(W2, dtype=np.float32).reshape(4, 2, P, 64)
    w2sb = np.ascontiguousarray(
        w2p.transpose(2, 0, 1, 3).reshape(P, 8 * 64)
    ).astype(np.float16)

    # W3 [256,255] -> [P, 2, 255]
    w3p = np.asarray(W3, dtype=np.float32).reshape(2, P, N_OUT)
    w3sb = np.ascontiguousarray(
        w3p.transpose(1, 0, 2).reshape(P, 2 * N_OUT)
    ).astype(np.float16)

    # biases (per-partition layouts)
    b1m = np.asarray(b1, dtype=np.float32).mean(axis=0)        # [16,64]
    b1t = np.ascontiguousarray(b1m.reshape(N_PAIR, P).T)       # [128, 8]
    b2t = np.ascontiguousarray(np.asarray(b2, dtype=np.float32).T)  # [64, 4]
    b3t = np.ascontiguousarray(
        np.broadcast_to(np.asarray(b3, dtype=np.float32), (P, N_OUT))
    )                                                          # [128, 255]
    return w1sb, w2sb, w3sb, b1t, b2t, b3t


def build_kernel(reps=1, has_bias=False, mode="full", unroll_hint=8):
    nc = bacc.Bacc("TRN2", target_bir_lowering=False, debug=False,
                   num_devices=N_CORES)
    f16 = mybir.dt.float16
    f32 = mybir.dt.float32

    x_ext = nc.declare_dram_parameter(
        "x", [N_CHUNK, KT_ALL * P * NB], f16, isOutput=False)
    w1_ext = nc.declare_dram_parameter("w1", [P, N_BLK * P], f16, isOutput=False)
    w2_ext = nc.declare_dram_parameter("w2", [P, 8 * 64], f16, isOutput=False)
    w3_ext = nc.declare_dram_parameter("w3", [P, 2 * N_OUT], f16, isOutput=False)
    b1_ext = nc.declare_dram_parameter("b1t", [P, N_PAIR], f32, isOutput=False)
    b2_ext = nc.declare_dram_parameter("b2t", [64, 4], f32, isOutput=False)
    b3_ext = nc.declare_dram_parameter("b3t", [P, N_OUT], f32, isOutput=False)
    out_ext = nc.declare_dram_parameter("out", [B_SHARD, N_OUT], f32, isOutput=True)

    with tile.TileContext(nc) as tc:
        with (
            tc.tile_pool(name="wpool", bufs=1) as wpool,
            tc.tile_pool(name="xt", bufs=2) as xt_pool,
            tc.tile_pool(name="hp", bufs=16) as hp_pool,
            tc.tile_pool(name="gt", bufs=2) as gt_pool,
            tc.tile_pool(name="osb", bufs=2) as out_pool,
            tc.tile_pool(name="ps1", bufs=2, space="PSUM") as ps1_pool,
            tc.tile_pool(name="ps2", bufs=2, space="PSUM") as ps2_pool,
            tc.tile_pool(name="ps3", bufs=2, space="PSUM") as ps3_pool,
        ):
            w1sb = wpool.tile([P, N_BLK, P], f16)
            nc.scalar.dma_start(out=w1sb[:], in_=w1_ext.rearrange("p (b j) -> p b j", j=P))
            w2sb = wpool.tile([P, 8, 64], f16)
            nc.scalar.dma_start(out=w2sb[:], in_=w2_ext.rearrange("p (b j) -> p b j", j=64))
            w3sb = wpool.tile([P, 2, N_OUT], f16)
            nc.scalar.dma_start(out=w3sb[:], in_=w3_ext.rearrange("p (b j) -> p b j", j=N_OUT))
            b1sb = wpool.tile([P, N_PAIR], f32)
            nc.scalar.dma_start(out=b1sb[:], in_=b1_ext[:])
            b2sb = wpool.tile([64, 4], f32)
            nc.scalar.dma_start(out=b2sb[:], in_=b2_ext[:])
            b3sb = wpool.tile([P, N_OUT], f32)
            nc.scalar.dma_start(out=b3sb[:], in_=b3_ext[:])

            env = locals()
            if reps == 1:
                _kernel_body(nc, tc, env, has_bias, mode)
            else:
                # one body outside the loop + For_i over (reps-1)/U ticks of
                # U bodies each -- amortizes the all-engine back-edge barrier
                # and the PE back-edge I$ miss (branch hint armed for PE).
                _kernel_body(nc, tc, env, has_bias, mode)
                unroll = next(
                    u for u in (unroll_hint, 8, 4, 2, 1) if (reps - 1) % u == 0
                )
                with tc.For_i(0, (reps - 1) // unroll, 1,
                              hint_engines=(mybir.EngineType.PE,)):
                    for _ in range(unroll):
                        _kernel_body(nc, tc, env, has_bias, mode)

    nc.compile()
    return nc


def _kernel_body(nc, tc, env, has_bias, mode="full"):
    x_ext = env["x_ext"]
    out_ext = env["out_ext"]
    w1sb, w2sb, w3sb = env["w1sb"], env["w2sb"], env["w3sb"]
    b1sb, b2sb, b3sb = env["b1sb"], env["b2sb"], env["b3sb"]
    xt_pool = env["xt_pool"]
    hp_pool, gt_pool, out_pool = env["hp_pool"], env["gt_pool"], env["out_pool"]
    ps1_pool, ps2_pool, ps3_pool = env["ps1_pool"], env["ps2_pool"], env["ps3_pool"]
    f16 = mybir.dt.float16
    f32 = mybir.dt.float32

    if mode == "none":
        # barrier-cost probe: one tiny DVE op per body
        scr = hp_pool.tile([P, 8], f32, name="scr")
        nc.vector.tensor_copy(out=scr[:], in_=env["b1sb"][:])
        return

    b0 = 0
    for ch in range(N_CHUNK):
        nb = NB
        # xt [128k, 48(q=t*3+c), nb]; staged sub-DMAs over contiguous q ranges
        # so layer-1 matmuls on early pairs start before the chunk has landed.
        xt_t = xt_pool.tile([P, KT_ALL, NB], f16, name="xtt")
        xt = xt_t[:, :, :nb]
        # host layout is [ch][p][q][b]: each partition's bytes are contiguous
        # in DRAM, so every (partition, q-group) run is one fat descriptor.
        xsrc = x_ext[ch].rearrange("(p q b) -> p q b", q=KT_ALL, p=P)
        if mode in ("full", "dma"):
            for (q0, q1) in QGROUPS:
                nc.sync.dma_start(out=xt[:, q0:q1, :], in_=xsrc[:, q0:q1, :])
        elif mode in ("compute", "l1"):
            # compute-only probes: tiny write so Tile sees the tile
            # initialized (values are garbage; timing-only probe)
            nc.sync.dma_start(out=xt[:, 0:1, :128], in_=xsrc[:, 0:1, :128])

        if mode == "dma":
            # out-store DMA on garbage osb to keep queue contention realistic
            osb_t = out_pool.tile([P, NB // P, N_OUT], f32, name="osbt")
            nc.vector.tensor_copy(out=osb_t[:, 0, 0:8], in_=b3sb[:, 0:8])
            nc.scalar.dma_start(
                out=out_ext[b0:b0 + nb, :].rearrange("(j p) n -> p j n", p=P),
                in_=osb_t[:, :nb // P],
            )
            b0 += nb
            continue

        if mode == "l1":
            # pure layer-1 matmul-rate probe (no copies, no L2/L3, no DMA)
            for m in range(N_PAIR):
                ps_t = ps1_pool.tile([P, NB], f32, name="ps1t")
                ps = ps_t[:, :nb]
                mm_list = [(c, t) for c in range(N_CH) for t in _pair_tiles(m)]
                for i, (c, t) in enumerate(mm_list):
                    nc.tensor.matmul(
                        ps[:],
                        w1sb[:, BLK_IDX[(m, c, t)], :],
                        xt[:, 3 * t + c, :],
                        start=(i == 0),
                        stop=(i == len(mm_list) - 1),
                    )
            b0 += nb
            continue

        if mode == "l1w":
            # weight-reuse probe: consecutive MM pairs share lhsT (two rhs
            # halves of the chunk) -- tests whether LDWEIGHTS gets elided.
            for m in range(N_PAIR):
                psa_t = ps1_pool.tile([P, NB // 2], f32, name="ps1a")
                psb_t = ps1_pool.tile([P, NB // 2], f32, name="ps1b")
                mm_list = [(c, t) for c in range(N_CH) for t in _pair_tiles(m)]
                for i, (c, t) in enumerate(mm_list):
                    w = w1sb[:, BLK_IDX[(m, c, t)], :]
                    nc.tensor.matmul(
                        psa_t[:], w, xt[:, 3 * t + c, :NB // 2],
                        start=(i == 0), stop=(i == len(mm_list) - 1),
                    )
                    nc.tensor.matmul(
                        psb_t[:], w, xt[:, 3 * t + c, NB // 2:],
                        start=(i == 0), stop=(i == len(mm_list) - 1),
                    )
            b0 += nb
            continue

        # ---- fused layers, interleaved to shorten the serial tail ----
        # pairs 0-3 -> L2 g0,g1 -> pair 4 -> L3 piece0 (all js, start)
        #   -> pairs 5-7 -> L2 g2,g3 -> L3 piece1 (stop) -> copies -> out DMA
        hps = {}
        gt_t = gt_pool.tile([P, 2, NB], f16, name="gtt")
        gt = gt_t[:, :, :nb]
        osb_t = out_pool.tile([P, NB // P, N_OUT], f32, name="osbt")
        osb = osb_t[:, :nb // P]
        # two js outputs share one PSUM bank ([P, 2, 256] f32 = 2 KiB)
        ps3b = [
            ps3_pool.tile([P, 2, 256], f32, name=f"ps3t{i}")
            for i in range(nb // P // 2)
        ]
        ps3s = [ps3b[js // 2][:, js % 2, :N_OUT] for js in range(nb // P)]

        def l1_pair(m):
            ps_t = ps1_pool.tile([P, NB], f32, name="ps1t")
            ps = ps_t[:, :nb]
            mm_list = [(c, t) for c in range(N_CH) for t in _pair_tiles(m)]
            for i, (c, t) in enumerate(mm_list):
                nc.tensor.matmul(
                    ps[:],
                    w1sb[:, BLK_IDX[(m, c, t)], :],
                    xt[:, 3 * t + c, :],
                    start=(i == 0),
                    stop=(i == len(mm_list) - 1),
                )
            hp_t = hp_pool.tile([P, NB], f16, name="hpt")
            hp = hp_t[:, :nb]
            if has_bias:
                if m % 2 == 0:
                    nc.vector.tensor_scalar_add(hp[:], ps[:], b1sb[:, m:m + 1])
                else:
                    nc.scalar.add(hp[:], ps[:], b1sb[:, m:m + 1])
            else:
                if m % 2 == 0:
                    nc.vector.tensor_copy(out=hp[:], in_=ps[:])
                else:
                    nc.scalar.copy(out=hp[:], in_=ps[:])
            hps[m] = hp

        def l2_group(g):
            ps2_t = ps2_pool.tile([64, NB], f32, name="ps2t")
            ps2 = ps2_t[:, :nb]
            for piece in range(2):
                nc.tensor.matmul(
                    ps2[:],
                    w2sb[:, 2 * g + piece, :],
                    hps[2 * g + piece][:],
                    start=(piece == 0),
                    stop=(piece == 1),
                )
            lo = 64 * (g % 2)
            if has_bias:
                nc.vector.tensor_scalar_add(
                    gt[lo:lo + 64, g // 2], ps2[:], b2sb[:, g:g + 1],
                )
            else:
                nc.vector.tensor_copy(out=gt[lo:lo + 64, g // 2], in_=ps2[:])

        def l3_piece(piece):
            # two js share one PSUM bank; start=True clears the WHOLE bank,
            # so only the bank's first MM may set it (the second js's piece-0
            # lands on cleared has_written bits -> overwrite+set, correct).
            for js in range(nb // P):
                nc.tensor.matmul(
                    ps3s[js],
                    gt[:, piece, js * P:(js + 1) * P],
                    w3sb[:, piece, :],
                    start=(piece == 0 and js % 2 == 0),
                    stop=(piece == 1),
                    skip_group_check=True,
                )

        for m in range(4):
            l1_pair(m)
        l2_group(0)
        l2_group(1)
        l1_pair(4)
        l3_piece(0)
        for m in range(5, N_PAIR):
            l1_pair(m)
        l2_group(2)
        l2_group(3)
        l3_piece(1)

        for js in range(nb // P):
            if has_bias:
                nc.vector.tensor_tensor(
                    osb[:, js], ps3s[js], b3sb[:], mybir.AluOpType.add,
                )
            else:
                nc.vector.tensor_copy(out=osb[:, js], in_=ps3s[js])
            if js % 2 == 1:
                nc.scalar.dma_start(
                    out=out_ext[b0 + (js - 1) * P:b0 + (js + 1) * P, :]
                        .rearrange("(j p) n -> p j n", p=P),
                    in_=osb[:, js - 1:js + 1],
                )
        b0 += nb


_CACHED_NC = None
_CACHED_BIAS_NC = None


def _prep_in_maps(x, W1, b1, W2, b2, W3, b3):
    # Host-side repack: [B, 6144] f32 -> per-core [2, 128, 16, 3, 512] f16
    # (chunk, p, t, c, b) -- partition-contiguous so each (p, q-group) DMA
    # run is one fat multi-KB descriptor.
    x16 = np.asarray(x, dtype=np.float16)
    w1sb, w2sb, w3sb, b1t, b2t, b3t = _pack_weights(W1, b1, W2, b2, W3, b3)
    in_maps = []
    for i in range(N_CORES):
        xc = x16[i * B_SHARD:(i + 1) * B_SHARD]
        xc = xc.reshape(N_CHUNK, NB, N_CH, KT_CH, P).transpose(0, 4, 3, 2, 1)
        xc = np.ascontiguousarray(xc).reshape(N_CHUNK, KT_ALL * P * NB)
        in_maps.append({
            "x": xc,
            "w1": w1sb,
            "w2": w2sb,
            "w3": w3sb,
            "b1t": b1t,
            "b2t": b2t,
            "b3t": b3t,
        })
    return in_maps


def kernel(x, W1, b1, W2, b2, W3, b3):
    global _CACHED_NC, _CACHED_BIAS_NC
    has_bias = bool(
        np.any(np.asarray(b1)) or np.any(np.asarray(b2)) or np.any(np.asarray(b3))
    )
    if has_bias:
        if _CACHED_BIAS_NC is None:
            _CACHED_BIAS_NC = build_kernel(has_bias=True)
        nc = _CACHED_BIAS_NC
    else:
        if _CACHED_NC is None:
            _CACHED_NC = build_kernel()
        nc = _CACHED_NC
    in_maps = _prep_in_maps(x, W1, b1, W2, b2, W3, b3)
    last_err = None
    for attempt in range(3):
        try:
            res = run_bass_kernel_spmd(nc, in_maps, core_ids=list(range(N_CORES)))
            break
        except Exception as e:  # transient device/axon failures
            last_err = e
            if attempt == 2:
                raise
            import time as _time
            _time.sleep(20.0)
    return np.concatenate([res.results[i]["out"] for i in range(N_CORES)], axis=0)
